# revision 5
# baseline (speedup 1.0000x reference)
"""MoE hard-routing kernel for Trainium2 (8 NeuronCores, Bass/Tile).

Problem: out[t] = x[t] @ W[p[t]].T + b[p[t]]
  x [8, 4096, 512] f32, partitions [8, 4096] int32 (values 0..7),
  W [8, 512, 512] f32, b [8, 512] f32.

Strategy: expert-parallel sharding. n_experts == n_cores == 8, so core e
owns expert e. The host routes each token to its expert's core (that IS the
shard assignment — a partition of the token set), pre-transposed so d_in
lies on SBUF partitions. Each core then runs one dense GEMM
  out_e[d_out, tok] = W[e] @ xT_e  (+ b[e])
accumulated over 4 K-chunks of 128 in PSUM, with the bias added during
PSUM eviction. Padding columns (zeros) are computed and discarded on the
host side during unsharding.
"""

import sys

for _p in ("/opt/trn_rl_repo", "/root/.axon_site/_ro/trn_rl_repo"):
    if _p not in sys.path:
        sys.path.append(_p)

import numpy as np

import concourse.bass as bass
import concourse.mybir as mybir
import concourse.tile as tile
from concourse.bass import ts
from concourse.bass_utils import run_bass_kernel_spmd
import bass_rust as _br

D_IN = 512
D_OUT = 512
N_EXPERTS = 8
N_CORES = 8
P = 128
NBLK = 512  # token columns per matmul (one PSUM bank of fp32)
KC = D_IN // P  # 4 contraction chunks
MC = D_OUT // P  # 4 output-row chunks

# "f32r" streams fp32 data through the PE at full rate (vs 4 cycles/row for
# exact fp32); "f32" is the exact-but-4x-slower fallback.
MATH_MODE = "f32r"


def _split_multiwait(nc: bass.Bass) -> None:
    """Hoist extra sem waits onto injected same-engine nops.

    The walrus build in this container rejects more than one sync-wait
    command on a single instruction.  Engine queues are in-order, so a
    nop carrying one wait immediately before the instruction is
    semantically identical to the wait being attached directly.
    """
    cnt = 0
    for bb in nc.main_func.blocks:
        new = []
        changed = False
        for ins in bb.instructions:
            si = ins.sync_info
            if si is not None and len(si.on_wait) > 1:
                waits = list(si.on_wait)
                for w in waits[:-1]:
                    nop = mybir.InstNoOp(name=f"wsplit-{cnt}", ins=[], outs=[])
                    cnt += 1
                    nop.engine = ins.engine
                    nop.sync_info = _br.SyncInfo(on_wait=[w], on_update=[])
                    new.append(nop)
                ins.sync_info = _br.SyncInfo(
                    on_wait=[waits[-1]], on_update=list(si.on_update)
                )
                changed = True
            new.append(ins)
        if changed:
            bb.instructions = new


def _build_nc(C: int, math_mode: str) -> bass.Bass:
    """One core's program: out[512, C] = wT.T-contract(xT) + bias."""
    f32 = mybir.dt.float32
    nc = bass.Bass("TRN2", target_bir_lowering=False, debug=False, num_devices=N_CORES)

    if math_mode == "f32r":
        mm_dt = mybir.dt.float32r
    elif math_mode == "f32":
        mm_dt = f32
    else:
        raise ValueError(math_mode)

    # Declaring the fp32r operands as fp32r in DRAM keeps the loads on the
    # fast HWDGE path (a f32->f32r casting DMA would be SWDGE-only, and the
    # BIR verifier requires matmul fp32r operands to be produced as fp32r).
    # The PE's own hi/lo decomposition of raw fp32 bits matches what the
    # rounding cast would produce to within half an ulp of the lo part.
    xT = nc.declare_dram_parameter("xT", [D_IN, C], mm_dt, isOutput=False)
    wT = nc.declare_dram_parameter("wT", [D_IN, D_OUT], mm_dt, isOutput=False)
    bias = nc.declare_dram_parameter("bias", [D_OUT], f32, isOutput=False)
    out = nc.declare_dram_parameter("out", [D_OUT, C], f32, isOutput=True)

    nblocks = C // NBLK
    with tile.TileContext(nc) as tc:
        with (
            tc.tile_pool(name="wpool", bufs=1) as wpool,
            tc.tile_pool(name="xpool", bufs=4) as xpool,
            tc.tile_pool(name="opool", bufs=3) as opool,
            tc.tile_pool(name="pspool", bufs=8, space="PSUM") as pspool,
        ):
            # Weights: wT[d_in, d_out] -> [128, KC, 512]; chunk (k, m) is the
            # stationary operand [K=128, M=128].  Loaded per k-chunk so the
            # first matmuls do not wait on the whole weight transfer.
            w_t = wpool.tile([P, KC, D_OUT], mm_dt)
            for k in range(KC):
                nc.sync.dma_start(
                    w_t[:, k, :], wT[ts(k, P), :].rearrange("p m -> p m")
                )
            # bias[d_out] -> [128, MC]; column m is the per-partition bias of
            # output-row chunk m.
            b_t = wpool.tile([P, MC], f32)
            nc.sync.dma_start(b_t[:], bias.rearrange("(m p) -> p m", p=P))

            for n in range(nblocks):
                x_t = xpool.tile([P, KC, NBLK], mm_dt)
                nc.sync.dma_start(
                    x_t[:],
                    xT[:, ts(n, NBLK)].rearrange("(k p) t -> p k t", p=P),
                )
                o_t = opool.tile([P, MC, NBLK], f32)
                for m in range(MC):
                    ps = pspool.tile([P, NBLK], f32)
                    for k in range(KC):
                        nc.tensor.matmul(
                            ps[:],
                            w_t[:, k, ts(m, P)],
                            x_t[:, k, :],
                            start=(k == 0),
                            stop=(k == KC - 1),
                        )
                    if m % 2 == 0:
                        nc.vector.tensor_scalar_add(
                            o_t[:, m, :], ps[:], b_t[:, m : m + 1]
                        )
                    else:
                        nc.scalar.activation(
                            o_t[:, m, :],
                            ps[:],
                            mybir.ActivationFunctionType.Identity,
                            bias=b_t[:, m : m + 1],
                        )
                nc.sync.dma_start(
                    out[:, ts(n, NBLK)].rearrange("(m p) t -> p m t", p=P),
                    o_t[:],
                )
    _split_multiwait(nc)
    return nc


_NC_CACHE: dict = {}


def _get_nc(C: int, math_mode: str) -> bass.Bass:
    key = (C, math_mode)
    if key not in _NC_CACHE:
        _NC_CACHE[key] = _build_nc(C, math_mode)
    return _NC_CACHE[key]


def kernel(x: np.ndarray, partitions: np.ndarray, W: np.ndarray, b: np.ndarray,
           _math_mode: str | None = None, _trace: bool = False):
    math_mode = _math_mode or MATH_MODE
    B, S, d_in = x.shape
    n_exp, d_out, _ = W.shape
    assert d_in == D_IN and d_out == D_OUT and n_exp == N_EXPERTS

    xf = np.ascontiguousarray(x, dtype=np.float32).reshape(-1, d_in)
    p = partitions.reshape(-1)

    tok_ids = [np.nonzero(p == e)[0] for e in range(N_EXPERTS)]
    max_cnt = max(len(ids) for ids in tok_ids)
    C = max(NBLK, ((max_cnt + NBLK - 1) // NBLK) * NBLK)

    in_maps = []
    for e in range(N_EXPERTS):
        ids = tok_ids[e]
        xT = np.zeros((D_IN, C), np.float32)
        xT[:, : len(ids)] = xf[ids].T
        in_maps.append(
            {
                "xT": xT,
                "wT": np.ascontiguousarray(W[e].T, dtype=np.float32),
                "bias": np.ascontiguousarray(b[e], dtype=np.float32),
            }
        )

    nc = _get_nc(C, math_mode)
    res = run_bass_kernel_spmd(nc, in_maps, list(range(N_CORES)), trace=_trace)

    outf = np.empty((B * S, d_out), np.float32)
    for e in range(N_EXPERTS):
        ids = tok_ids[e]
        outf[ids] = res.results[e]["out"][:, : len(ids)].T
    out = outf.reshape(B, S, d_out)
    if _trace:
        return out, res
    return out


# revision 6
# speedup vs baseline: 1.4185x; 1.4185x over previous
"""MoE hard-routing kernel for Trainium2 (8 NeuronCores, Bass/Tile).

Problem: out[t] = x[t] @ W[p[t]].T + b[p[t]]
  x [8, 4096, 512] f32, partitions [8, 4096] int32 (values 0..7),
  W [8, 512, 512] f32, b [8, 512] f32.

Strategy: expert-parallel sharding. n_experts == n_cores == 8, so core e
owns expert e. The host routes each token to its expert's core (that IS the
shard assignment — a partition of the token set), pre-transposed so d_in
lies on SBUF partitions. Each core then runs one dense GEMM
  out_e[d_out, tok] = W[e] @ xT_e  (+ b[e])
accumulated over 4 K-chunks of 128 in PSUM, with the bias added during
PSUM eviction. Padding columns (zeros) are computed and discarded on the
host side during unsharding.
"""

import sys

for _p in ("/opt/trn_rl_repo", "/root/.axon_site/_ro/trn_rl_repo"):
    if _p not in sys.path:
        sys.path.append(_p)

import numpy as np

import concourse.bass as bass
import concourse.mybir as mybir
import concourse.tile as tile
from concourse.bass import ts
from concourse.bass_utils import run_bass_kernel_spmd
import bass_rust as _br

D_IN = 512
D_OUT = 512
N_EXPERTS = 8
N_CORES = 8
P = 128
NBLK = 512  # token columns per matmul (one PSUM bank of fp32)
KC = D_IN // P  # 4 contraction chunks
MC = D_OUT // P  # 4 output-row chunks

# "f32r" streams fp32 data through the PE at full rate (vs 4 cycles/row for
# exact fp32); "f32" is the exact-but-4x-slower fallback.
MATH_MODE = "f32r"


def _split_multiwait(nc: bass.Bass) -> None:
    """Hoist extra sem waits onto injected same-engine nops.

    The walrus build in this container rejects more than one sync-wait
    command on a single instruction.  Engine queues are in-order, so a
    nop carrying one wait immediately before the instruction is
    semantically identical to the wait being attached directly.
    """
    cnt = 0
    for bb in nc.main_func.blocks:
        new = []
        changed = False
        for ins in bb.instructions:
            si = ins.sync_info
            if si is not None and len(si.on_wait) > 1:
                waits = list(si.on_wait)
                for w in waits[:-1]:
                    nop = mybir.InstNoOp(name=f"wsplit-{cnt}", ins=[], outs=[])
                    cnt += 1
                    nop.engine = ins.engine
                    nop.sync_info = _br.SyncInfo(on_wait=[w], on_update=[])
                    new.append(nop)
                ins.sync_info = _br.SyncInfo(
                    on_wait=[waits[-1]], on_update=list(si.on_update)
                )
                changed = True
            new.append(ins)
        if changed:
            bb.instructions = new


def _build_nc(C: int, math_mode: str) -> bass.Bass:
    """One core's program: out[512, C] = wT.T-contract(xT) + bias."""
    f32 = mybir.dt.float32
    nc = bass.Bass("TRN2", target_bir_lowering=False, debug=False, num_devices=N_CORES)

    if math_mode == "f32r":
        mm_dt = mybir.dt.float32r
    elif math_mode == "f32":
        mm_dt = f32
    else:
        raise ValueError(math_mode)

    # Declaring the fp32r operands as fp32r in DRAM keeps the loads on the
    # fast HWDGE path (a f32->f32r casting DMA would be SWDGE-only, and the
    # BIR verifier requires matmul fp32r operands to be produced as fp32r).
    # The PE's own hi/lo decomposition of raw fp32 bits matches what the
    # rounding cast would produce to within half an ulp of the lo part.
    xT = nc.declare_dram_parameter("xT", [D_IN, C], mm_dt, isOutput=False)
    wT = nc.declare_dram_parameter("wT", [D_IN, D_OUT], mm_dt, isOutput=False)
    bias = nc.declare_dram_parameter("bias", [D_OUT], f32, isOutput=False)
    out = nc.declare_dram_parameter("out", [D_OUT, C], f32, isOutput=True)

    nblocks = C // NBLK
    with tile.TileContext(nc) as tc:
        with (
            tc.tile_pool(name="wpool", bufs=1) as wpool,
            tc.tile_pool(name="xpool", bufs=4) as xpool,
            tc.tile_pool(name="opool", bufs=3) as opool,
            tc.tile_pool(name="pspool", bufs=8, space="PSUM") as pspool,
        ):
            # Weights: wT[d_in, d_out] -> [128, KC, 512]; chunk (k, m) is the
            # stationary operand [K=128, M=128].  Loaded per k-chunk so the
            # first matmuls do not wait on the whole weight transfer.
            w_t = wpool.tile([P, KC, D_OUT], mm_dt)
            for k in range(KC):
                nc.scalar.dma_start(w_t[:, k, :], wT[ts(k, P), :])
            # bias[d_out] -> [128, MC]; column m is the per-partition bias of
            # output-row chunk m.
            b_t = wpool.tile([P, MC], f32)
            nc.scalar.dma_start(b_t[:], bias.rearrange("(m p) -> p m", p=P))

            for n in range(nblocks):
                x_t = xpool.tile([P, KC, NBLK], mm_dt)
                nc.sync.dma_start(
                    x_t[:],
                    xT[:, ts(n, NBLK)].rearrange("(k p) t -> p k t", p=P),
                )
                o_t = opool.tile([P, MC, NBLK], f32)
                for m in range(MC):
                    ps = pspool.tile([P, NBLK], f32)
                    for k in range(KC):
                        nc.tensor.matmul(
                            ps[:],
                            w_t[:, k, ts(m, P)],
                            x_t[:, k, :],
                            start=(k == 0),
                            stop=(k == KC - 1),
                        )
                    if m % 2 == 0:
                        nc.vector.tensor_scalar_add(
                            o_t[:, m, :], ps[:], b_t[:, m : m + 1]
                        )
                    else:
                        nc.scalar.activation(
                            o_t[:, m, :],
                            ps[:],
                            mybir.ActivationFunctionType.Identity,
                            bias=b_t[:, m : m + 1],
                        )
                # Stores ride the ACT HWDGE ring so they don't serialize
                # behind the x loads on the SP ring.
                nc.scalar.dma_start(
                    out[:, ts(n, NBLK)].rearrange("(m p) t -> p m t", p=P),
                    o_t[:],
                )
    _split_multiwait(nc)
    return nc


_NC_CACHE: dict = {}


def _get_nc(C: int, math_mode: str) -> bass.Bass:
    key = (C, math_mode)
    if key not in _NC_CACHE:
        _NC_CACHE[key] = _build_nc(C, math_mode)
    return _NC_CACHE[key]


def kernel(x: np.ndarray, partitions: np.ndarray, W: np.ndarray, b: np.ndarray,
           _math_mode: str | None = None, _trace: bool = False):
    math_mode = _math_mode or MATH_MODE
    B, S, d_in = x.shape
    n_exp, d_out, _ = W.shape
    assert d_in == D_IN and d_out == D_OUT and n_exp == N_EXPERTS

    xf = np.ascontiguousarray(x, dtype=np.float32).reshape(-1, d_in)
    p = partitions.reshape(-1)

    tok_ids = [np.nonzero(p == e)[0] for e in range(N_EXPERTS)]
    max_cnt = max(len(ids) for ids in tok_ids)
    C = max(NBLK, ((max_cnt + NBLK - 1) // NBLK) * NBLK)

    in_maps = []
    for e in range(N_EXPERTS):
        ids = tok_ids[e]
        xT = np.zeros((D_IN, C), np.float32)
        xT[:, : len(ids)] = xf[ids].T
        in_maps.append(
            {
                "xT": xT,
                "wT": np.ascontiguousarray(W[e].T, dtype=np.float32),
                "bias": np.ascontiguousarray(b[e], dtype=np.float32),
            }
        )

    nc = _get_nc(C, math_mode)
    res = run_bass_kernel_spmd(nc, in_maps, list(range(N_CORES)), trace=_trace)

    outf = np.empty((B * S, d_out), np.float32)
    for e in range(N_EXPERTS):
        ids = tok_ids[e]
        outf[ids] = res.results[e]["out"][:, : len(ids)].T
    out = outf.reshape(B, S, d_out)
    if _trace:
        return out, res
    return out
